# revision 1
# baseline (speedup 1.0000x reference)
"""CartesianMACE rank-0 fused kernel for 8 trn2 NeuronCores.

The reference's ranks 1 and 2 never reach the output (each rank is mixed
independently and the head reads only h[0]), so only the rank-0 slices of
cw0/mw0/cw1/mw1 plus h0/msg0_r0/msg1_r0/w_pred/b_pred are needed.

Per node n (16x16 mats A=cw0[0,n], B=mw0[0,n], D=cw1[0,n], E=mw1[0,n];
16-vecs x=h0[n], m0=msg0_r0[n], m1=msg1_r0[n]):
    s[n] = colsum(D) . (A x + B m0) + colsum(E) . m1
    out  = [sum_n s[n] w_pred[0,n], sum_n s[n] w_pred[1,n]] + b_pred

Sharding: data-parallel over nodes. 50000 nodes padded to 50176 =
8 cores x 7 supertiles x 128 partitions x 7 groups. Nodes live on SBUF
partitions; the 256-element flattened matrices live on the free axis.
All compute on the vector engine; per-core (128,2) partials are summed
on host (the final all-reduce of the head).
"""

import sys
import types

for _p in ("/opt/trn_rl_repo", "/root/.axon_site/_ro/trn_rl_repo"):
    if _p not in sys.path:
        sys.path.append(_p)

import numpy as np

N, CH = 50000, 16
CORES = 8
T, S = 7, 7          # supertiles per core, groups per supertile
GP = T * S           # 49 groups of 128 nodes per core
NP = CORES * T * 128 * S  # 50176 padded nodes

_cache = {}
TRACE = False
GP_MUL2 = True  # run the B*m0 mult on GpSimd
GP_MUL1 = False  # set by test harness to capture an NTFF profile


def _split_multiwait(nc, mybir):
    """This walrus build accepts a single sync-wait per instruction, but Tile
    attaches one wait per producer proc. Split: keep the last wait on the
    instruction and hoist the rest onto fresh same-engine Drain carriers
    inserted immediately before it (engines execute their stream in-order,
    so semantics are identical)."""
    for fn in nc.m.functions:
        for bb in fn.blocks:
            insts = bb.instructions  # live list
            i = 0
            while i < len(insts):
                ins = insts[i]
                si = ins.sync_info
                if si is not None and len(si.on_wait) > 1:
                    waits = list(si.on_wait)
                    ins.sync_info = mybir.SyncInfo(
                        on_wait=waits[-1:], on_update=list(si.on_update))
                    for k, w in enumerate(waits[:-1]):
                        insts.insert(i + k, mybir.InstDrain(
                            name=f"{ins.name}_w{k}", opcode="Drain",
                            engine=ins.engine, ins=[], outs=[],
                            sync_info=mybir.SyncInfo(on_wait=[w], on_update=[]),
                        ))
                    i += len(waits) - 1
                i += 1


def _build_nc():
    import concourse.bass as bass
    import concourse.tile as tile
    import concourse.mybir as mybir

    f32 = mybir.dt.float32
    P = 128

    nc = bass.Bass("TRN2", target_bir_lowering=False, debug=False,
                   num_devices=CORES)

    ab_d = nc.dram_tensor("ab", [T, P, S * 512], f32,
                          kind="ExternalInput").ap()
    de_d = nc.dram_tensor("de", [T, P, S * 512], f32,
                          kind="ExternalInput").ap()
    xm_d = nc.dram_tensor("xm", [P, T * S * 32], f32,
                          kind="ExternalInput").ap()
    m1_d = nc.dram_tensor("m1", [P, T * S * 16], f32,
                          kind="ExternalInput").ap()
    w_d = nc.dram_tensor("w", [P, 2 * GP], f32, kind="ExternalInput").ap()
    o_d = nc.dram_tensor("o", [P, 2], f32, kind="ExternalOutput").ap()

    with tile.TileContext(nc) as tc:
        with (
            tc.tile_pool(name="mats", bufs=4) as mats,
            tc.tile_pool(name="vecs", bufs=3) as vecs,
            tc.tile_pool(name="work", bufs=2) as work,
            tc.tile_pool(name="acc", bufs=1) as acc,
        ):
            # persistent accumulators, finalized after the loop
            tvm_all = acc.tile([P, 2 * GP * 16], f32)   # [tA | tB] row sums
            deq_all = acc.tile([P, 2 * GP * 16], f32)   # [d | e] colsums
            v_all = acc.tile([P, 2 * GP * 16], f32)     # [tv | m1]
            w_sb = acc.tile([P, 2 * GP], f32)
            xm_all = acc.tile([P, T * S * 32], f32)
            nc.sync.dma_start(out=xm_all[:, :], in_=xm_d)

            for t in range(T):
                ab_sb = mats.tile([P, S * 512], f32, tag="ab")
                nc.sync.dma_start(out=ab_sb[:, 0:S * 256],
                                  in_=ab_d[t][:, 0:S * 256])
                nc.sync.dma_start(out=ab_sb[:, S * 256:S * 512],
                                  in_=ab_d[t][:, S * 256:S * 512])
                de_sb = mats.tile([P, S * 512], f32, tag="de")
                nc.sync.dma_start(out=de_sb[:, :], in_=de_d[t])
                xm_sb = xm_all[:, t * S * 32:(t + 1) * S * 32]

                # tmp[m,g,j,k] = {A,B}[g,j,k] * {x,m0}[g,k]
                gjk = lambda ap: ap.rearrange("p (g j k) -> p g j k",
                                              g=S, j=16, k=16)
                bc = lambda ap: (ap.rearrange("p (g k) -> p g k", g=S, k=16)
                                 .unsqueeze(2).broadcast_to((P, S, 16, 16)))
                H = S * 256
                tmp = work.tile([P, S * 512], f32, tag="tmp")
                tmp5 = tmp[:, :].rearrange("p (m g j k) -> p m g j k",
                                           m=2, g=S, j=16, k=16)
                MUL1 = nc.gpsimd if GP_MUL1 else nc.vector
                MUL1.tensor_mul(out=gjk(tmp[:, 0:H]),
                                in0=gjk(ab_sb[:, 0:H]),
                                in1=bc(xm_sb[:, 0:S * 16]))
                MUL2 = nc.gpsimd if GP_MUL2 else nc.vector
                MUL2.tensor_mul(out=gjk(tmp[:, H:2 * H]),
                                in0=gjk(ab_sb[:, H:2 * H]),
                                in1=bc(xm_sb[:, S * 16:S * 32]))

                # row sums into tvm_all[:, m, t, g, j]
                nc.vector.reduce_sum(
                    out=tvm_all[:, :].rearrange("p (m t g j) -> p m t g j",
                                                m=2, t=T, g=S, j=16)[:, :, t],
                    in_=tmp5, axis=mybir.AxisListType.X)

                # colsums: D,E host-transposed (k-major), j contiguous.
                # GpSimd folds j 16->8, DVE reduces the remaining 8.
                h8 = work.tile([P, S * 256], f32, tag="h8")
                de4 = de_sb[:, :].rearrange("p (q k j) -> p q k j",
                                            q=2 * S, k=16, j=16)
                nc.gpsimd.tensor_add(
                    out=h8[:, :].rearrange("p (q k j) -> p q k j",
                                           q=2 * S, k=16, j=8),
                    in0=de4[:, :, :, 0:8], in1=de4[:, :, :, 8:16])
                nc.vector.reduce_sum(
                    out=deq_all[:, :].rearrange("p (m t g k) -> p m t g k",
                                                m=2, t=T, g=S, k=16)[:, :, t],
                    in_=h8[:, :].rearrange("p (m g k j) -> p m g k j",
                                           m=2, g=S, k=16, j=8),
                    axis=mybir.AxisListType.X)

            # ---- epilogue: all the small per-group math, once ----
            nc.sync.dma_start(out=v_all[:, GP * 16:2 * GP * 16], in_=m1_d)
            nc.sync.dma_start(out=w_sb[:, :], in_=w_d)
            nc.vector.tensor_add(out=v_all[:, 0:GP * 16],
                                 in0=tvm_all[:, 0:GP * 16],
                                 in1=tvm_all[:, GP * 16:2 * GP * 16])
            pr = acc.tile([P, 2 * GP * 16], f32)
            nc.vector.tensor_mul(out=pr[:, :], in0=deq_all[:, :],
                                 in1=v_all[:, :])
            sm = acc.tile([P, 2 * GP], f32)
            nc.vector.reduce_sum(
                out=sm[:, :].rearrange("p (m tg) -> p m tg", m=2, tg=GP),
                in_=pr[:, :].rearrange("p (m tg k) -> p m tg k",
                                       m=2, tg=GP, k=16),
                axis=mybir.AxisListType.X)
            s_all = acc.tile([P, GP], f32)
            nc.vector.tensor_add(out=s_all[:, :], in0=sm[:, 0:GP],
                                 in1=sm[:, GP:2 * GP])
            # head: o[:, c] = sum_g s_all[:, g] * w[:, c*GP+g]
            junk = acc.tile([P, 2 * GP], f32)
            nc.vector.tensor_mul(
                out=junk[:, :].rearrange("p (c g) -> p c g", c=2, g=GP),
                in0=s_all[:, :].rearrange("p g -> p g").unsqueeze(1)
                .broadcast_to((P, 2, GP)),
                in1=w_sb[:, :].rearrange("p (c g) -> p c g", c=2, g=GP))
            o_sb = acc.tile([P, 2], f32)
            nc.vector.reduce_sum(
                out=o_sb[:, :].rearrange("p c -> p c"),
                in_=junk[:, :].rearrange("p (c g) -> p c g", c=2, g=GP),
                axis=mybir.AxisListType.X)
            nc.sync.dma_start(out=o_d, in_=o_sb[:, :])

    return nc


def _get_nc():
    if "nc" not in _cache:
        _cache["nc"] = _build_nc()
    return _cache["nc"]


def _shard_mat(m):
    """(N,16,16) -> (CORES, T, 128, S*256), zero-padded, group-major free axis."""
    out = np.zeros((NP, 256), np.float32)
    out[:N] = np.asarray(m, np.float32).reshape(N, 256)
    return np.ascontiguousarray(out.reshape(CORES, T, 128, S * 256))


def _shard_vec(v):
    """(N,16) -> (CORES, T, 128, S*16)."""
    out = np.zeros((NP, 16), np.float32)
    out[:N] = np.asarray(v, np.float32).reshape(N, 16)
    return np.ascontiguousarray(out.reshape(CORES, T, 128, S * 16))


def kernel(h0, cw0, mw0, cw1, mw1,
           msg0_r0, msg0_r1, msg0_r2,
           msg1_r0, msg1_r1, msg1_r2,
           w_pred, b_pred):
    from concourse.bass_utils import run_bass_kernel_spmd

    nc = _get_nc()
    if not _cache.get("split_done"):
        import concourse.mybir as mybir
        _split_multiwait(nc, mybir)
        _cache["split_done"] = True

    A4 = _shard_mat(cw0[0]).reshape(CORES, T, 128, S, 256)
    B4 = _shard_mat(mw0[0]).reshape(CORES, T, 128, S, 256)
    AB = np.ascontiguousarray(
        np.stack([A4, B4], axis=3).reshape(CORES, T, 128, S * 512))
    DE = np.ascontiguousarray(
        np.stack([_shard_mat(np.swapaxes(np.asarray(cw1[0], np.float32), 1, 2))
                  .reshape(CORES, T, 128, S, 256),
                  _shard_mat(np.swapaxes(np.asarray(mw1[0], np.float32), 1, 2))
                  .reshape(CORES, T, 128, S, 256)],
                 axis=3).reshape(CORES, T, 128, S * 512))
    X = _shard_vec(np.asarray(h0, np.float32)[..., 0])
    M0 = _shard_vec(np.asarray(msg0_r0, np.float32)[..., 0])
    XM = np.ascontiguousarray(
        np.stack([X, M0], axis=3).reshape(CORES, T, 128, S * 32)
        .transpose(0, 2, 1, 3).reshape(CORES, 128, T * S * 32))
    M1 = np.ascontiguousarray(
        _shard_vec(np.asarray(msg1_r0, np.float32)[..., 0])
        .transpose(0, 2, 1, 3).reshape(CORES, 128, T * S * 16))

    wp = np.zeros((2, NP), np.float32)
    wp[:, :N] = np.asarray(w_pred, np.float32)
    # (2, CORES, T, 128, S) -> (CORES, 128, 2, T, S) -> (CORES, 128, 2*GP)
    W = np.ascontiguousarray(
        wp.reshape(2, CORES, T, 128, S).transpose(1, 3, 0, 2, 4)
        .reshape(CORES, 128, 2 * GP))

    in_maps = [
        {"ab": AB[i], "de": DE[i], "xm": XM[i], "m1": M1[i], "w": W[i]}
        for i in range(CORES)
    ]
    res = run_bass_kernel_spmd(nc, in_maps, list(range(CORES)), trace=TRACE)
    _cache["last_res"] = res
    partial = np.zeros(2, np.float64)
    for i in range(CORES):
        partial += res.results[i]["o"].astype(np.float64).sum(axis=0)
    out = (partial + np.asarray(b_pred, np.float64)).astype(np.float32)
    return out.reshape(1, 2)



# revision 2
# speedup vs baseline: 1.2274x; 1.2274x over previous
"""CartesianMACE rank-0 fused kernel for 8 trn2 NeuronCores — v2 (fp16, all-DVE).

The reference's ranks 1 and 2 never reach the output (each rank is mixed
independently and the head reads only h[0]), so only the rank-0 slices of
cw0/mw0/cw1/mw1 plus h0/msg0_r0/msg1_r0/w_pred/b_pred are needed.

Per node n (16x16 mats A=cw0[0,n], B=mw0[0,n], D=cw1[0,n], E=mw1[0,n];
16-vecs x=h0[n], m0=msg0_r0[n], m1=msg1_r0[n]):
    s[n] = colsum(D) . (A x + B m0) + colsum(E) . m1
    out  = [sum_n s[n] w_pred[0,n], sum_n s[n] w_pred[1,n]] + b_pred

v2 design (from trace analysis of v1):
  * fp16 end-to-end: halves HBM traffic (the bottleneck: ~358 GB/s/core)
    and enables DVE 2x_1P mode on tensor_tensor ops.
  * All compute on the Vector engine. v1 ran muls at half rate whenever
    GpSimd was active (shared SBUF port pair); with GpSimd idle every TT
    op gets its full rate.
  * tensor_reduce is capped at 1x regardless of dtype, so all 16->1
    segment sums are log2 fold trees of tensor_tensor adds (2x mode),
    with the tmp and DE trees merged after stage 1.
  * Everything fits in SBUF at fp16 (~150 KiB/partition), so all DMAs
    are issued up front on the sync HWDGE ring (FIFO) and compute
    chases the chunks; no buffer recycling stalls.

Sharding: data-parallel over nodes. 50000 nodes padded to 50176 =
8 cores x 7 supertiles x 128 partitions x 7 groups. Per-core (128,2)
partials are summed on host (the final all-reduce of the head).
"""

import sys

for _p in ("/opt/trn_rl_repo", "/root/.axon_site/_ro/trn_rl_repo"):
    if _p not in sys.path:
        sys.path.append(_p)

import numpy as np

N, CH = 50000, 16
CORES = 8
T, S = 7, 7          # supertiles per core, node groups per partition row
P = 128
GP = T * S           # 49 groups of 128 nodes per core
NP = CORES * T * P * S  # 50176 padded nodes

_cache = {}
TRACE = False


def _split_multiwait(nc, mybir):
    """This walrus build accepts a single sync-wait per instruction, but Tile
    attaches one wait per producer proc. Split: keep the last wait on the
    instruction and hoist the rest onto fresh same-engine Drain carriers
    inserted immediately before it (engines execute their stream in-order,
    so semantics are identical)."""
    for fn in nc.m.functions:
        for bb in fn.blocks:
            insts = bb.instructions  # live list
            i = 0
            while i < len(insts):
                ins = insts[i]
                si = ins.sync_info
                if si is not None and len(si.on_wait) > 1:
                    waits = list(si.on_wait)
                    ins.sync_info = mybir.SyncInfo(
                        on_wait=waits[-1:], on_update=list(si.on_update))
                    for k, w in enumerate(waits[:-1]):
                        insts.insert(i + k, mybir.InstDrain(
                            name=f"{ins.name}_w{k}", opcode="Drain",
                            engine=ins.engine, ins=[], outs=[],
                            sync_info=mybir.SyncInfo(on_wait=[w], on_update=[]),
                        ))
                    i += len(waits) - 1
                i += 1


def _build_nc():
    import concourse.bass as bass
    import concourse.tile as tile
    import concourse.mybir as mybir

    f16 = mybir.dt.float16
    f32 = mybir.dt.float32

    nc = bass.Bass("TRN2", target_bir_lowering=False, debug=False,
                   num_devices=CORES)

    # [g7][m2][j16][k16] per (t, p): m=0 -> A=cw0[0], m=1 -> B=mw0[0]
    ab_d = nc.dram_tensor("ab", [T, P, S * 512], f16, kind="ExternalInput").ap()
    # [g7][m2][k16][j16] per (t, p): m=0 -> D=cw1[0]^T, m=1 -> E=mw1[0]^T
    de_d = nc.dram_tensor("de", [T, P, S * 512], f16, kind="ExternalInput").ap()
    # [t7][g7][m2][k16]: m=0 -> x=h0, m=1 -> m0=msg0_r0
    xm_d = nc.dram_tensor("xm", [P, T * S * 32], f16, kind="ExternalInput").ap()
    # [t7][g7][k16]: msg1_r0
    m1_d = nc.dram_tensor("m1", [P, T * S * 16], f16, kind="ExternalInput").ap()
    # [c2][t7][g7]: w_pred
    w_d = nc.dram_tensor("w", [P, 2 * GP], f16, kind="ExternalInput").ap()
    o_d = nc.dram_tensor("o", [P, 2], f32, kind="ExternalOutput").ap()

    H = S * 256  # 1792: one m-slice of a chunk

    with tile.TileContext(nc) as tc:
        with (
            tc.tile_pool(name="acc", bufs=1) as acc,
            tc.tile_pool(name="work", bufs=2) as work,
        ):
            ab_sb = [acc.tile([P, 2 * H], f16, name=f"ab{t}") for t in range(T)]
            de_sb = [acc.tile([P, 2 * H], f16, name=f"de{t}") for t in range(T)]
            xm_sb = acc.tile([P, T * S * 32], f16)
            m1_sb = acc.tile([P, T * S * 16], f16)
            v_sb = acc.tile([P, T * S * 16], f16)
            # per chunk: [half2][g7][m2][x16]; half 0 = tvm (j-indexed row
            # sums of A,B), half 1 = deq (k-indexed colsums of D,E)
            sd_sb = acc.tile([P, T * 448], f16)
            w_sb = acc.tile([P, 2 * GP], f16)
            o_sb = acc.tile([P, 2], f32)

            # All loads up front on the sync HWDGE ring: FIFO completion
            # order == issue order, so compute chases the chunk stream.
            nc.sync.dma_start(out=xm_sb[:, :], in_=xm_d)
            for t in range(T):
                nc.sync.dma_start(out=ab_sb[t][:, :], in_=ab_d[t])
                nc.sync.dma_start(out=de_sb[t][:, :], in_=de_d[t])
            nc.sync.dma_start(out=m1_sb[:, :], in_=m1_d)
            nc.sync.dma_start(out=w_sb[:, :], in_=w_d)

            for t in range(T):
                # tmp[g,m,j,k] = {A,B}[g,m,j,k] * {x,m0}[g,m,k]
                tmp = work.tile([P, 2 * H], f16, tag="tmp")
                nc.vector.tensor_mul(
                    out=tmp[:, :].rearrange("p (g m j k) -> p g m j k",
                                            g=S, m=2, j=16, k=16),
                    in0=ab_sb[t][:, :].rearrange("p (g m j k) -> p g m j k",
                                                 g=S, m=2, j=16, k=16),
                    in1=xm_sb[:, t * S * 32:(t + 1) * S * 32]
                    .rearrange("p (g m k) -> p g m k", g=S, m=2, k=16)
                    .unsqueeze(3).broadcast_to((P, S, 2, 16, 16)))

                # merged fold trees: [tmp | de] halves fold their innermost
                # 16 (k for tmp -> tvm[g,m,j]; j for de -> deq[g,m,k])
                h1 = work.tile([P, 2 * H], f16, tag="h1")
                h1a = h1[:, :].rearrange("p (u n i) -> p u n i", u=2, n=224, i=8)
                srcs = (tmp, de_sb[t])
                for u in range(2):
                    sv = srcs[u][:, :].rearrange("p (n i) -> p n i", n=224, i=16)
                    nc.vector.tensor_add(out=h1a[:, u], in0=sv[:, :, 0:8],
                                         in1=sv[:, :, 8:16])
                h2 = work.tile([P, H], f16, tag="h2")
                h1b = h1[:, :].rearrange("p (n i) -> p n i", n=448, i=8)
                nc.vector.tensor_add(
                    out=h2[:, :].rearrange("p (n i) -> p n i", n=448, i=4),
                    in0=h1b[:, :, 0:4], in1=h1b[:, :, 4:8])
                h3 = work.tile([P, H // 2], f16, tag="h3")
                h2b = h2[:, :].rearrange("p (n i) -> p n i", n=448, i=4)
                nc.vector.tensor_add(
                    out=h3[:, :].rearrange("p (n i) -> p n i", n=448, i=2),
                    in0=h2b[:, :, 0:2], in1=h2b[:, :, 2:4])
                h3b = h3[:, :].rearrange("p (n i) -> p n i", n=448, i=2)
                nc.vector.tensor_add(out=sd_sb[:, t * 448:(t + 1) * 448],
                                     in0=h3b[:, :, 0], in1=h3b[:, :, 1])

            # ---- epilogue ----
            sd5 = sd_sb[:, :].rearrange("p (t h g m x) -> p t h g m x",
                                        t=T, h=2, g=S, m=2, x=16)
            # v[t,g,j] = tA + tB
            nc.vector.tensor_add(
                out=v_sb[:, :].rearrange("p (t g x) -> p t g x",
                                         t=T, g=S, x=16),
                in0=sd5[:, :, 0, :, 0], in1=sd5[:, :, 0, :, 1])
            # pr = [d*v | e*m1]  (both 784-elem halves)
            pr = acc.tile([P, 2 * T * S * 16], f16)
            pr4 = pr[:, :].rearrange("p (q t g x) -> p q t g x",
                                     q=2, t=T, g=S, x=16)
            nc.vector.tensor_mul(
                out=pr4[:, 0], in0=sd5[:, :, 1, :, 0],
                in1=v_sb[:, :].rearrange("p (t g x) -> p t g x",
                                         t=T, g=S, x=16))
            nc.vector.tensor_mul(
                out=pr4[:, 1], in0=sd5[:, :, 1, :, 1],
                in1=m1_sb[:, :].rearrange("p (t g x) -> p t g x",
                                          t=T, g=S, x=16))
            # fold pr's innermost 16 -> s2[q, tg]
            g1 = acc.tile([P, GP * 16], f16)
            prb = pr[:, :].rearrange("p (n i) -> p n i", n=2 * GP, i=16)
            nc.vector.tensor_add(
                out=g1[:, :].rearrange("p (n i) -> p n i", n=2 * GP, i=8),
                in0=prb[:, :, 0:8], in1=prb[:, :, 8:16])
            g2 = acc.tile([P, GP * 8], f16)
            g1b = g1[:, :].rearrange("p (n i) -> p n i", n=2 * GP, i=8)
            nc.vector.tensor_add(
                out=g2[:, :].rearrange("p (n i) -> p n i", n=2 * GP, i=4),
                in0=g1b[:, :, 0:4], in1=g1b[:, :, 4:8])
            g3 = acc.tile([P, GP * 4], f16)
            g2b = g2[:, :].rearrange("p (n i) -> p n i", n=2 * GP, i=4)
            nc.vector.tensor_add(
                out=g3[:, :].rearrange("p (n i) -> p n i", n=2 * GP, i=2),
                in0=g2b[:, :, 0:2], in1=g2b[:, :, 2:4])
            s2 = acc.tile([P, GP * 2], f16)
            g3b = g3[:, :].rearrange("p (n i) -> p n i", n=2 * GP, i=2)
            nc.vector.tensor_add(out=s2[:, :], in0=g3b[:, :, 0],
                                 in1=g3b[:, :, 1])
            # s[tg] = s_d + s_e
            s_all = acc.tile([P, GP], f16)
            s2b = s2[:, :].rearrange("p (q n) -> p q n", q=2, n=GP)
            nc.vector.tensor_add(out=s_all[:, :], in0=s2b[:, 0],
                                 in1=s2b[:, 1])
            # head: o[c] = sum_tg s[tg] * w[c, tg]
            hp = acc.tile([P, 2 * GP], f16)
            nc.vector.tensor_mul(
                out=hp[:, :].rearrange("p (c n) -> p c n", c=2, n=GP),
                in0=w_sb[:, :].rearrange("p (c n) -> p c n", c=2, n=GP),
                in1=s_all[:, :].unsqueeze(1).broadcast_to((P, 2, GP)))
            nc.vector.reduce_sum(
                out=o_sb[:, :],
                in_=hp[:, :].rearrange("p (c n) -> p c n", c=2, n=GP),
                axis=mybir.AxisListType.X)
            nc.sync.dma_start(out=o_d, in_=o_sb[:, :])

    return nc


def _get_nc():
    if "nc" not in _cache:
        _cache["nc"] = _build_nc()
    return _cache["nc"]


def _pack_mats(M0, M1, transpose):
    """Two (N,16,16) f32 mats -> (CORES, T, 128, 3584) f16, [g][m][256]."""
    a = np.zeros((NP, 2, 256), np.float16)
    m0 = np.asarray(M0, np.float32)
    m1 = np.asarray(M1, np.float32)
    if transpose:
        m0 = m0.swapaxes(1, 2)
        m1 = m1.swapaxes(1, 2)
    a[:N, 0] = m0.reshape(N, 256).astype(np.float16)
    a[:N, 1] = m1.reshape(N, 256).astype(np.float16)
    return np.ascontiguousarray(a.reshape(CORES, T, P, S * 512))


def kernel(h0, cw0, mw0, cw1, mw1,
           msg0_r0, msg0_r1, msg0_r2,
           msg1_r0, msg1_r1, msg1_r2,
           w_pred, b_pred):
    from concourse.bass_utils import run_bass_kernel_spmd

    nc = _get_nc()
    if not _cache.get("split_done"):
        import concourse.mybir as mybir
        _split_multiwait(nc, mybir)
        _cache["split_done"] = True

    AB = _pack_mats(cw0[0], mw0[0], transpose=False)
    DE = _pack_mats(cw1[0], mw1[0], transpose=True)

    xm = np.zeros((NP, 2, 16), np.float16)
    xm[:N, 0] = np.asarray(h0, np.float32)[..., 0].astype(np.float16)
    xm[:N, 1] = np.asarray(msg0_r0, np.float32)[..., 0].astype(np.float16)
    XM = np.ascontiguousarray(
        xm.reshape(CORES, T, P, S * 32).transpose(0, 2, 1, 3)
        .reshape(CORES, P, T * S * 32))

    m1 = np.zeros((NP, 16), np.float16)
    m1[:N] = np.asarray(msg1_r0, np.float32)[..., 0].astype(np.float16)
    M1 = np.ascontiguousarray(
        m1.reshape(CORES, T, P, S * 16).transpose(0, 2, 1, 3)
        .reshape(CORES, P, T * S * 16))

    wp = np.zeros((2, NP), np.float32)
    wp[:, :N] = np.asarray(w_pred, np.float32)
    W = np.ascontiguousarray(
        wp.reshape(2, CORES, T, P, S).transpose(1, 3, 0, 2, 4)
        .reshape(CORES, P, 2 * GP).astype(np.float16))

    in_maps = [
        {"ab": AB[i], "de": DE[i], "xm": XM[i], "m1": M1[i], "w": W[i]}
        for i in range(CORES)
    ]
    res = run_bass_kernel_spmd(nc, in_maps, list(range(CORES)), trace=TRACE)
    _cache["last_res"] = res
    partial = np.zeros(2, np.float64)
    for i in range(CORES):
        partial += res.results[i]["o"].astype(np.float64).sum(axis=0)
    out = (partial + np.asarray(b_pred, np.float64)).astype(np.float32)
    return out.reshape(1, 2)


# revision 6
# speedup vs baseline: 1.4951x; 1.2181x over previous
"""CartesianMACE rank-0 fused kernel for 8 trn2 NeuronCores — v3.

The reference's ranks 1 and 2 never reach the output (each rank is mixed
independently and the head reads only h[0]), so only the rank-0 slices of
cw0/mw0/cw1/mw1 plus h0/msg0_r0/msg1_r0/w_pred/b_pred are needed.

Per node n (16x16 mats A=cw0[0,n], B=mw0[0,n], D=cw1[0,n], E=mw1[0,n];
16-vecs x=h0[n], m0=msg0_r0[n], m1=msg1_r0[n]):
    s[n] = colsum(D) . (A x + B m0) + colsum(E) . m1
    out  = [sum_n s[n] w_pred[0,n], sum_n s[n] w_pred[1,n]] + b_pred

v3 design (evolved from v2 trace analysis):
  * fp16 end-to-end (halves HBM traffic, enables DVE 2x_1P tensor_tensor).
  * All compute on the Vector engine (GpSimd idle -> no shared-port
    stalls); reductions are log2 fold trees of TT adds, not tensor_reduce
    (which is capped at 1x).
  * g-innermost free layout [m,j,k,g] / [m,k,j,g]: fold-tree operands are
    long contiguous runs (56/28/14 elems) instead of 8/4/2/1, removing
    most per-run AP-walk overhead seen in the v2 trace.
  * The two fold trees share stages 2-4 (h1 holds both trees' stage-1
    outputs side by side).
  * Everything resident in SBUF; all DMAs issued up front on the sync
    HWDGE ring (FIFO), compute chases the chunks.

Sharding: data-parallel over nodes. 50000 nodes padded to 50176 =
8 cores x 7 supertiles x 128 partitions x 7 groups. Per-core (128,2)
partials are summed on host (the final all-reduce of the head).
"""

import sys

for _p in ("/opt/trn_rl_repo", "/root/.axon_site/_ro/trn_rl_repo"):
    if _p not in sys.path:
        sys.path.append(_p)

import numpy as np

N, CH = 50000, 16
CORES = 8
T, S = 7, 7          # supertiles per core, node groups per partition row
P = 128
GP = T * S           # 49 groups of 128 nodes per core
NP = CORES * T * P * S  # 50176 padded nodes

_cache = {}
TRACE = False


def _split_multiwait(nc, mybir):
    """This walrus build accepts a single sync-wait per instruction, but Tile
    attaches one wait per producer proc. Split: keep the last wait on the
    instruction and hoist the rest onto fresh same-engine Drain carriers
    inserted immediately before it (engines execute their stream in-order,
    so semantics are identical)."""
    for fn in nc.m.functions:
        for bb in fn.blocks:
            insts = bb.instructions  # live list
            i = 0
            while i < len(insts):
                ins = insts[i]
                si = ins.sync_info
                if si is not None and len(si.on_wait) > 1:
                    waits = list(si.on_wait)
                    ins.sync_info = mybir.SyncInfo(
                        on_wait=waits[-1:], on_update=list(si.on_update))
                    for k, w in enumerate(waits[:-1]):
                        insts.insert(i + k, mybir.InstDrain(
                            name=f"{ins.name}_w{k}", opcode="Drain",
                            engine=ins.engine, ins=[], outs=[],
                            sync_info=mybir.SyncInfo(on_wait=[w], on_update=[]),
                        ))
                    i += len(waits) - 1
                i += 1


def _build_nc():
    import concourse.bass as bass
    import concourse.tile as tile
    import concourse.mybir as mybir

    f16 = mybir.dt.float16
    f32 = mybir.dt.float32

    nc = bass.Bass("TRN2", target_bir_lowering=False, debug=False,
                   num_devices=CORES)

    CK = 2 * 16 * 16 * S  # 3584 elems per chunk tensor

    # [m2][j16][k16][g7] per (t, p): m=0 -> A=cw0[0], m=1 -> B=mw0[0]
    ab_d = nc.dram_tensor("ab", [T, P, CK], f16, kind="ExternalInput").ap()
    # [m2][k16][j16][g7] per (t, p): m=0 -> D=cw1[0], m=1 -> E=mw1[0]
    de_d = nc.dram_tensor("de", [T, P, CK], f16, kind="ExternalInput").ap()
    # [t7][m2][k16][g7]: m=0 -> x=h0, m=1 -> m0=msg0_r0
    xm_d = nc.dram_tensor("xm", [P, T * 224], f16, kind="ExternalInput").ap()
    # [t7][k16][g7]: msg1_r0
    m1_d = nc.dram_tensor("m1", [P, T * 112], f16, kind="ExternalInput").ap()
    # [c2][t7][g7]: w_pred
    w_d = nc.dram_tensor("w", [P, 2 * GP], f16, kind="ExternalInput").ap()
    o_d = nc.dram_tensor("o", [P, 2], f32, kind="ExternalOutput").ap()

    with tile.TileContext(nc) as tc:
        with (
            tc.tile_pool(name="acc", bufs=1) as acc,
            tc.tile_pool(name="work", bufs=2) as work,
        ):
            ab_sb = [acc.tile([P, CK], f16, name=f"ab{t}") for t in range(T)]
            de_sb = [acc.tile([P, CK], f16, name=f"de{t}") for t in range(T)]
            xm_sb = acc.tile([P, T * 224], f16)
            m1_sb = acc.tile([P, T * 112], f16)
            v_sb = acc.tile([P, T * 112], f16)
            # per chunk: [tree2][m2][x16][g7]; tree 0 = tvm (j-indexed row
            # sums of A,B), tree 1 = deq (k-indexed colsums of D,E)
            sd_sb = acc.tile([P, T * 448], f16)
            w_sb = acc.tile([P, 2 * GP], f16)
            o_sb = acc.tile([P, 2], f32)

            # All loads up front on the sync HWDGE ring: FIFO completion
            # order == issue order, so compute chases the chunk stream.
            nc.sync.dma_start(out=xm_sb[:, :], in_=xm_d)
            for t in range(T):
                nc.sync.dma_start(out=ab_sb[t][:, :], in_=ab_d[t])
                nc.sync.dma_start(out=de_sb[t][:, :], in_=de_d[t])
            nc.sync.dma_start(out=m1_sb[:, :], in_=m1_d)
            nc.sync.dma_start(out=w_sb[:, :], in_=w_d)

            for t in range(T):
                # tmp[m,j,k,g] = {A,B}[m,j,k,g] * {x,m0}[m,k,g]  (bcast j)
                tmp = work.tile([P, CK], f16, tag="tmp")
                xm4 = (xm_sb[:, t * 224:(t + 1) * 224]
                       .rearrange("p (m k g) -> p m k g", m=2, k=16, g=S)
                       .unsqueeze(2).broadcast_to((P, 2, 16, 16, S)))
                nc.vector.tensor_mul(
                    out=tmp[:, :].rearrange("p (m j k g) -> p m j k g",
                                            m=2, j=16, k=16, g=S),
                    in0=ab_sb[t][:, :].rearrange("p (m j k g) -> p m j k g",
                                                 m=2, j=16, k=16, g=S),
                    in1=xm4)

                # stage 1 of both fold trees -> h1 = [tmp-half | de-half],
                # then shared stages 2-4 fold 8 -> 1 (long g-runs).
                h1 = work.tile([P, CK], f16, tag="h1")
                h1v = h1[:, :].rearrange("p (u b r) -> p u b r",
                                         u=2, b=32, r=56)
                for u, src in enumerate((tmp, de_sb[t])):
                    sv = src[:, :].rearrange("p (b r) -> p b r", b=32, r=112)
                    nc.vector.tensor_add(out=h1v[:, u], in0=sv[:, :, 0:56],
                                         in1=sv[:, :, 56:112])
                h2 = work.tile([P, CK // 2], f16, tag="h2")
                h1b = h1[:, :].rearrange("p (b r) -> p b r", b=64, r=56)
                nc.vector.tensor_add(
                    out=h2[:, :].rearrange("p (b r) -> p b r", b=64, r=28),
                    in0=h1b[:, :, 0:28], in1=h1b[:, :, 28:56])
                h3 = work.tile([P, CK // 4], f16, tag="h3")
                h2b = h2[:, :].rearrange("p (b r) -> p b r", b=64, r=28)
                nc.vector.tensor_add(
                    out=h3[:, :].rearrange("p (b r) -> p b r", b=64, r=14),
                    in0=h2b[:, :, 0:14], in1=h2b[:, :, 14:28])
                h3b = h3[:, :].rearrange("p (b r) -> p b r", b=64, r=14)
                nc.vector.tensor_add(
                    out=sd_sb[:, t * 448:(t + 1) * 448]
                    .rearrange("p (b r) -> p b r", b=64, r=7),
                    in0=h3b[:, :, 0:7], in1=h3b[:, :, 7:14])

            # ---- epilogue ----
            sd6 = sd_sb[:, :].rearrange("p (t u m x g) -> p t u m x g",
                                        t=T, u=2, m=2, x=16, g=S)
            # v[t,x,g] = tA + tB
            nc.vector.tensor_add(
                out=v_sb[:, :].rearrange("p (t x g) -> p t x g",
                                         t=T, x=16, g=S),
                in0=sd6[:, :, 0, 0], in1=sd6[:, :, 0, 1])
            # pr = [d*v | e*m1]
            pr = acc.tile([P, 2 * T * 112], f16)
            pr4 = pr[:, :].rearrange("p (q t x g) -> p q t x g",
                                     q=2, t=T, x=16, g=S)
            nc.vector.tensor_mul(
                out=pr4[:, 0], in0=sd6[:, :, 1, 0],
                in1=v_sb[:, :].rearrange("p (t x g) -> p t x g",
                                         t=T, x=16, g=S))
            nc.vector.tensor_mul(
                out=pr4[:, 1], in0=sd6[:, :, 1, 1],
                in1=m1_sb[:, :].rearrange("p (t x g) -> p t x g",
                                          t=T, x=16, g=S))
            # fold pr's x16 -> s2[q,t,g]
            g1 = acc.tile([P, T * 112], f16)
            prb = pr[:, :].rearrange("p (b r) -> p b r", b=2 * T, r=112)
            nc.vector.tensor_add(
                out=g1[:, :].rearrange("p (b r) -> p b r", b=2 * T, r=56),
                in0=prb[:, :, 0:56], in1=prb[:, :, 56:112])
            g2 = acc.tile([P, T * 56], f16)
            g1b = g1[:, :].rearrange("p (b r) -> p b r", b=2 * T, r=56)
            nc.vector.tensor_add(
                out=g2[:, :].rearrange("p (b r) -> p b r", b=2 * T, r=28),
                in0=g1b[:, :, 0:28], in1=g1b[:, :, 28:56])
            g3 = acc.tile([P, T * 28], f16)
            g2b = g2[:, :].rearrange("p (b r) -> p b r", b=2 * T, r=28)
            nc.vector.tensor_add(
                out=g3[:, :].rearrange("p (b r) -> p b r", b=2 * T, r=14),
                in0=g2b[:, :, 0:14], in1=g2b[:, :, 14:28])
            s2 = acc.tile([P, T * 14], f16)
            g3b = g3[:, :].rearrange("p (b r) -> p b r", b=2 * T, r=14)
            nc.vector.tensor_add(
                out=s2[:, :].rearrange("p (b r) -> p b r", b=2 * T, r=7),
                in0=g3b[:, :, 0:7], in1=g3b[:, :, 7:14])
            # s[t,g] = s_d + s_e
            s_all = acc.tile([P, GP], f16)
            s2b = s2[:, :].rearrange("p (q n) -> p q n", q=2, n=GP)
            nc.vector.tensor_add(out=s_all[:, :], in0=s2b[:, 0],
                                 in1=s2b[:, 1])
            # head: o[c] = sum_tg s[tg] * w[c, tg]
            hp = acc.tile([P, 2 * GP], f16)
            nc.vector.tensor_mul(
                out=hp[:, :].rearrange("p (c n) -> p c n", c=2, n=GP),
                in0=w_sb[:, :].rearrange("p (c n) -> p c n", c=2, n=GP),
                in1=s_all[:, :].unsqueeze(1).broadcast_to((P, 2, GP)))
            nc.vector.reduce_sum(
                out=o_sb[:, :],
                in_=hp[:, :].rearrange("p (c n) -> p c n", c=2, n=GP),
                axis=mybir.AxisListType.X)
            nc.sync.dma_start(out=o_d, in_=o_sb[:, :])

    return nc


def _get_nc():
    if "nc" not in _cache:
        _cache["nc"] = _build_nc()
    return _cache["nc"]


def _pack_mats(M0, M1, transpose):
    """Two (N,16,16) f32 mats -> (CORES, T, 128, 3584) f16, [m][16][16][g]."""
    a = np.zeros((NP, 2, 16, 16), np.float16)
    m0 = np.asarray(M0, np.float32)
    m1 = np.asarray(M1, np.float32)
    if transpose:
        m0 = m0.swapaxes(1, 2)
        m1 = m1.swapaxes(1, 2)
    a[:N, 0] = m0.astype(np.float16)
    a[:N, 1] = m1.astype(np.float16)
    # [c,t,p,g,m,y,z] -> [c,t,p,m,y,z,g]
    return np.ascontiguousarray(
        a.reshape(CORES, T, P, S, 2, 16, 16).transpose(0, 1, 2, 4, 5, 6, 3)
        .reshape(CORES, T, P, 2 * 256 * S))


def kernel(h0, cw0, mw0, cw1, mw1,
           msg0_r0, msg0_r1, msg0_r2,
           msg1_r0, msg1_r1, msg1_r2,
           w_pred, b_pred):
    from concourse.bass_utils import run_bass_kernel_spmd

    nc = _get_nc()
    if not _cache.get("split_done"):
        import concourse.mybir as mybir
        _split_multiwait(nc, mybir)
        _cache["split_done"] = True

    AB = _pack_mats(cw0[0], mw0[0], transpose=False)
    DE = _pack_mats(cw1[0], mw1[0], transpose=True)

    xm = np.zeros((NP, 2, 16), np.float16)
    xm[:N, 0] = np.asarray(h0, np.float32)[..., 0].astype(np.float16)
    xm[:N, 1] = np.asarray(msg0_r0, np.float32)[..., 0].astype(np.float16)
    # [c,t,p,g,m,k] -> [c,p,t,m,k,g]
    XM = np.ascontiguousarray(
        xm.reshape(CORES, T, P, S, 2, 16).transpose(0, 2, 1, 4, 5, 3)
        .reshape(CORES, P, T * 224))

    m1 = np.zeros((NP, 16), np.float16)
    m1[:N] = np.asarray(msg1_r0, np.float32)[..., 0].astype(np.float16)
    # [c,t,p,g,k] -> [c,p,t,k,g]
    M1 = np.ascontiguousarray(
        m1.reshape(CORES, T, P, S, 16).transpose(0, 2, 1, 4, 3)
        .reshape(CORES, P, T * 112))

    wp = np.zeros((2, NP), np.float32)
    wp[:, :N] = np.asarray(w_pred, np.float32)
    W = np.ascontiguousarray(
        wp.reshape(2, CORES, T, P, S).transpose(1, 3, 0, 2, 4)
        .reshape(CORES, P, 2 * GP).astype(np.float16))

    in_maps = [
        {"ab": AB[i], "de": DE[i], "xm": XM[i], "m1": M1[i], "w": W[i]}
        for i in range(CORES)
    ]
    res = run_bass_kernel_spmd(nc, in_maps, list(range(CORES)), trace=TRACE)
    _cache["last_res"] = res
    partial = np.zeros(2, np.float64)
    for i in range(CORES):
        partial += res.results[i]["o"].astype(np.float64).sum(axis=0)
    out = (partial + np.asarray(b_pred, np.float64)).astype(np.float32)
    return out.reshape(1, 2)
